# revision 3
# baseline (speedup 1.0000x reference)
"""Trainium2 Bass kernel for per-channel batched Linear:
    out[b,c,p,e] = sum_q W[e,p,q] * x[b,c,q,e] + bias[e,p]

Shapes: x [16,128,512,64] f32, W [64,512,512] f32, b [64,512] f32.

Strategy: shard embed_dim E=64 across 8 cores (8 channels/core). Each
channel is an independent GEMM out_e^T = W_e @ X_e^T with X_e = x[..,..,:,e]
flattened to [2048, 512]. We compute the transposed output [P, M] so the
weight is the matmul's stationary operand (reused across the M dim) and the
bias becomes a per-partition scalar for the PSUM->SBUF eviction op.

Host side: slice + cast fp32 -> bf16 + lay out operands so every DMA is a
big contiguous 128-partition transfer. Device: bf16 matmuls (full PE rate),
fp32 PSUM accumulate, bias added during PSUM eviction on ACT/DVE.
"""

import numpy as np
import ml_dtypes
from contextlib import ExitStack

import concourse.bass as bass
import concourse.tile as tile
from concourse import bacc, mybir
from concourse import bass_utils
from concourse.bass import ts

B, C, Q, E = 16, 128, 512, 64
P = 512            # output projection size (== Q here)
N_CORES = 8
E_LOC = E // N_CORES   # 8 channels per core
M = B * C              # 2048 rows per channel GEMM

QT = Q // 128          # 4 k-tiles
PT = P // 128          # 4 output-partition tiles
MC = M // 512          # 4 moving-dim chunks

BF16 = mybir.dt.bfloat16
F32 = mybir.dt.float32

_CACHE = {}


def _kernel_body(tc, out, xt, wt, bias_d):
    nc = tc.nc
    with ExitStack() as ctx:
        xpool = ctx.enter_context(tc.tile_pool(name="x", bufs=3))
        wpool = ctx.enter_context(tc.tile_pool(name="w", bufs=3))
        opool = ctx.enter_context(tc.tile_pool(name="o", bufs=6))
        bpool = ctx.enter_context(tc.tile_pool(name="bias", bufs=1))
        psum = ctx.enter_context(tc.tile_pool(name="psum", bufs=2, space="PSUM"))

        bias_sb = bpool.tile([128, E_LOC * PT], F32)
        nc.sync.dma_start(bias_sb[:], bias_d[:])

        for e in range(E_LOC):
            # whole channel of W^T first (small, needed by every matmul)
            w_sb = wpool.tile([128, QT, P], BF16, tag="w")
            nc.sync.dma_start(
                w_sb[:], wt[e].rearrange("(qt qp) p -> qp qt p", qp=128)
            )
            # channel of X^T, one DMA per m-chunk so compute can start after
            # the first 512 KiB instead of the full 2 MiB
            x_sb = xpool.tile([128, QT, M], BF16, tag="x")
            xt_r = xt[e].rearrange("(qt qp) m -> qp qt m", qp=128)
            for mc in range(MC):
                nc.sync.dma_start(
                    x_sb[:, :, ts(mc, 512)], xt_r[:, :, ts(mc, 512)]
                )

            for pt in range(PT):
                bj = e * PT + pt
                bias_ap = bias_sb[:, bj : bj + 1]
                for mc in range(MC):
                    # mc-outer / qt-inner: each PSUM bank's accumulation
                    # group is 4 back-to-back matmuls, so bank mc evicts
                    # while bank mc+1 computes
                    ps = psum.tile([128, 512], F32, name=f"ps_{mc % 2}")
                    for qt in range(QT):
                        nc.tensor.matmul(
                            ps[:],
                            w_sb[:, qt, ts(pt, 128)],
                            x_sb[:, qt, ts(mc, 512)],
                            start=(qt == 0),
                            stop=(qt == QT - 1),
                        )
                    o_sb = opool.tile([128, 512], BF16, tag="o")
                    # alternate eviction between ACT and DVE so neither is
                    # the bottleneck
                    if mc % 2 == 0:
                        nc.scalar.activation(
                            o_sb[:],
                            ps[:],
                            mybir.ActivationFunctionType.Identity,
                            bias=bias_ap,
                        )
                    else:
                        nc.vector.tensor_scalar_add(o_sb[:], ps[:], bias_ap)
                    nc.sync.dma_start(
                        out[e, ts(pt, 128), ts(mc, 512)], o_sb[:]
                    )


def _build():
    if "nc" in _CACHE:
        return _CACHE["nc"]
    nc = bacc.Bacc(
        "TRN2",
        target_bir_lowering=False,
        debug=False,
        enable_asserts=True,
        num_devices=N_CORES,
        enable_partition_id=False,
    )
    xt = nc.dram_tensor("xt", [E_LOC, Q, M], BF16, kind="ExternalInput").ap()
    wt = nc.dram_tensor("wt", [E_LOC, Q, P], BF16, kind="ExternalInput").ap()
    bias_d = nc.dram_tensor("bias", [128, E_LOC * PT], F32, kind="ExternalInput").ap()
    out = nc.dram_tensor("out", [E_LOC, P, M], BF16, kind="ExternalOutput").ap()
    with tile.TileContext(nc) as tc:
        _kernel_body(tc, out, xt, wt, bias_d)
    nc.compile()
    _CACHE["nc"] = nc
    return nc


def make_in_maps(x, W, b):
    """Host-side shard + cast + layout. Returns list of 8 per-core dicts."""
    in_maps = []
    for r in range(N_CORES):
        e0 = r * E_LOC
        e1 = e0 + E_LOC
        # X^T per channel: [e, q, m] where m = b*C + c
        xs = (
            x[:, :, :, e0:e1]
            .transpose(3, 2, 0, 1)
            .astype(ml_dtypes.bfloat16)
            .reshape(E_LOC, Q, M)
        )
        # W^T per channel: [e, q, p]
        ws = np.ascontiguousarray(W[e0:e1].transpose(0, 2, 1)).astype(
            ml_dtypes.bfloat16
        )
        # bias laid out [128, e*PT + pt] with partition = p % 128
        bs = np.ascontiguousarray(
            b[e0:e1].reshape(E_LOC, PT, 128).transpose(2, 0, 1)
        ).reshape(128, E_LOC * PT)
        in_maps.append({"xt": xs, "wt": ws, "bias": np.ascontiguousarray(bs, np.float32)})
    return in_maps


def assemble_output(results):
    out = np.empty((B, C, P, E), np.float32)
    for r in range(N_CORES):
        o = results[r]["out"]  # [E_LOC, P, M] bf16
        out[:, :, :, r * E_LOC : (r + 1) * E_LOC] = (
            o.astype(np.float32).transpose(2, 1, 0).reshape(B, C, P, E_LOC)
        )
    return out


def run_on_hw(x, W, b, **spmd_kwargs):
    nc = _build()
    in_maps = make_in_maps(x, W, b)
    res = bass_utils.run_bass_kernel_spmd(
        nc, in_maps, core_ids=list(range(N_CORES)), **spmd_kwargs
    )
    return assemble_output(res.results), res


def kernel(x, W, b):
    out, _ = run_on_hw(x, W, b)
    return out


# revision 4
# speedup vs baseline: 1.2743x; 1.2743x over previous
"""Trainium2 Bass kernel for per-channel batched Linear:
    out[b,c,p,e] = sum_q W[e,p,q] * x[b,c,q,e] + bias[e,p]

Shapes: x [16,128,512,64] f32, W [64,512,512] f32, b [64,512] f32.

Strategy: shard embed_dim E=64 across 8 cores (8 channels/core). Each
channel is an independent GEMM out_e^T = W_e @ X_e^T with X_e = x[..,..,:,e]
flattened to [2048, 512]. We compute the transposed output [P, M] so the
weight is the matmul's stationary operand (reused across the M dim) and the
bias becomes a per-partition scalar for the PSUM->SBUF eviction op.

Host side: slice + cast fp32 -> bf16 + lay out operands so every DMA is a
big contiguous 128-partition transfer. Device: bf16 matmuls (full PE rate),
fp32 PSUM accumulate, bias added during PSUM eviction on ACT/DVE.
"""

import numpy as np
import ml_dtypes
from contextlib import ExitStack

import concourse.bass as bass
import concourse.tile as tile
from concourse import bacc, mybir
from concourse import bass_utils
from concourse.bass import ts

B, C, Q, E = 16, 128, 512, 64
P = 512            # output projection size (== Q here)
N_CORES = 8
E_LOC = E // N_CORES   # 8 channels per core
M = B * C              # 2048 rows per channel GEMM

QT = Q // 128          # 4 k-tiles
PT = P // 128          # 4 output-partition tiles
MC = M // 512          # 4 moving-dim chunks

BF16 = mybir.dt.bfloat16
F32 = mybir.dt.float32

_CACHE = {}


def _kernel_body(tc, out, xt, wt, bias_d):
    nc = tc.nc
    with ExitStack() as ctx:
        xpool = ctx.enter_context(tc.tile_pool(name="x", bufs=3))
        wpool = ctx.enter_context(tc.tile_pool(name="w", bufs=3))
        opool = ctx.enter_context(tc.tile_pool(name="o", bufs=6))
        bpool = ctx.enter_context(tc.tile_pool(name="bias", bufs=1))
        psum = ctx.enter_context(tc.tile_pool(name="psum", bufs=2, space="PSUM"))

        bias_sb = bpool.tile([128, E_LOC * PT], F32)
        nc.sync.dma_start(bias_sb[:], bias_d[:])

        for e in range(E_LOC):
            # whole channel of W^T first (small, needed by every matmul)
            w_sb = wpool.tile([128, QT, P], BF16, tag="w")
            nc.sync.dma_start(
                w_sb[:], wt[e].rearrange("(qt qp) p -> qp qt p", qp=128)
            )
            # channel of X^T, one DMA per m-chunk so compute can start after
            # the first 512 KiB instead of the full 2 MiB
            x_sb = xpool.tile([128, QT, M], BF16, tag="x")
            xt_r = xt[e].rearrange("(qt qp) m -> qp qt m", qp=128)
            for mc in range(MC):
                nc.sync.dma_start(
                    x_sb[:, :, ts(mc, 512)], xt_r[:, :, ts(mc, 512)]
                )

            for pt in range(PT):
                bj = e * PT + pt
                bias_ap = bias_sb[:, bj : bj + 1]
                o_sb = opool.tile([128, M], BF16, tag="o")
                for mc in range(MC):
                    # mc-outer / qt-inner: each PSUM bank's accumulation
                    # group is 4 back-to-back matmuls, so bank mc evicts
                    # while bank mc+1 computes
                    ps = psum.tile([128, 512], F32, name=f"ps_{mc % 2}")
                    for qt in range(QT):
                        nc.tensor.matmul(
                            ps[:],
                            w_sb[:, qt, ts(pt, 128)],
                            x_sb[:, qt, ts(mc, 512)],
                            start=(qt == 0),
                            stop=(qt == QT - 1),
                        )
                    # alternate eviction between ACT and DVE so neither is
                    # the bottleneck
                    if mc % 2 == 0:
                        nc.scalar.activation(
                            o_sb[:, ts(mc, 512)],
                            ps[:],
                            mybir.ActivationFunctionType.Identity,
                            bias=bias_ap,
                        )
                    else:
                        nc.vector.tensor_scalar_add(
                            o_sb[:, ts(mc, 512)], ps[:], bias_ap
                        )
                # out DMA rides the ACT HWDGE ring so its eviction-gated
                # wait never blocks the Sync ring's input prefetch (HWDGE
                # is FIFO per ring)
                nc.scalar.dma_start(out[e, ts(pt, 128)], o_sb[:])


def _build():
    if "nc" in _CACHE:
        return _CACHE["nc"]
    nc = bacc.Bacc(
        "TRN2",
        target_bir_lowering=False,
        debug=False,
        enable_asserts=True,
        num_devices=N_CORES,
        enable_partition_id=False,
    )
    xt = nc.dram_tensor("xt", [E_LOC, Q, M], BF16, kind="ExternalInput").ap()
    wt = nc.dram_tensor("wt", [E_LOC, Q, P], BF16, kind="ExternalInput").ap()
    bias_d = nc.dram_tensor("bias", [128, E_LOC * PT], F32, kind="ExternalInput").ap()
    out = nc.dram_tensor("out", [E_LOC, P, M], BF16, kind="ExternalOutput").ap()
    with tile.TileContext(nc) as tc:
        _kernel_body(tc, out, xt, wt, bias_d)
    nc.compile()
    _CACHE["nc"] = nc
    return nc


def make_in_maps(x, W, b):
    """Host-side shard + cast + layout. Returns list of 8 per-core dicts."""
    in_maps = []
    for r in range(N_CORES):
        e0 = r * E_LOC
        e1 = e0 + E_LOC
        # X^T per channel: [e, q, m] where m = b*C + c
        xs = (
            x[:, :, :, e0:e1]
            .transpose(3, 2, 0, 1)
            .astype(ml_dtypes.bfloat16)
            .reshape(E_LOC, Q, M)
        )
        # W^T per channel: [e, q, p]
        ws = np.ascontiguousarray(W[e0:e1].transpose(0, 2, 1)).astype(
            ml_dtypes.bfloat16
        )
        # bias laid out [128, e*PT + pt] with partition = p % 128
        bs = np.ascontiguousarray(
            b[e0:e1].reshape(E_LOC, PT, 128).transpose(2, 0, 1)
        ).reshape(128, E_LOC * PT)
        in_maps.append({"xt": xs, "wt": ws, "bias": np.ascontiguousarray(bs, np.float32)})
    return in_maps


def assemble_output(results):
    out = np.empty((B, C, P, E), np.float32)
    for r in range(N_CORES):
        o = results[r]["out"]  # [E_LOC, P, M] bf16
        out[:, :, :, r * E_LOC : (r + 1) * E_LOC] = (
            o.astype(np.float32).transpose(2, 1, 0).reshape(B, C, P, E_LOC)
        )
    return out


def run_on_hw(x, W, b, **spmd_kwargs):
    nc = _build()
    in_maps = make_in_maps(x, W, b)
    res = bass_utils.run_bass_kernel_spmd(
        nc, in_maps, core_ids=list(range(N_CORES)), **spmd_kwargs
    )
    return assemble_output(res.results), res


def kernel(x, W, b):
    out, _ = run_on_hw(x, W, b)
    return out


# revision 6
# speedup vs baseline: 1.3139x; 1.0311x over previous
"""Trainium2 Bass kernel for per-channel batched Linear:
    out[b,c,p,e] = sum_q W[e,p,q] * x[b,c,q,e] + bias[e,p]

Shapes: x [16,128,512,64] f32, W [64,512,512] f32, b [64,512] f32.

Strategy: shard embed_dim E=64 across 8 cores (8 channels/core). Each
channel is an independent GEMM out_e^T = W_e @ X_e^T with X_e = x[..,..,:,e]
flattened to [2048, 512]. We compute the transposed output [P, M] so the
weight is the matmul's stationary operand (reused across the M dim) and the
bias becomes a per-partition scalar for the PSUM->SBUF eviction op.

Host side: slice + cast fp32 -> bf16 + lay out operands so every DMA is a
big contiguous 128-partition transfer. Device: bf16 matmuls (full PE rate),
fp32 PSUM accumulate, bias added during PSUM eviction on ACT/DVE.
"""

import numpy as np
import ml_dtypes
from contextlib import ExitStack

import concourse.bass as bass
import concourse.tile as tile
from concourse import bacc, mybir
from concourse import bass_utils
from concourse.bass import ts

B, C, Q, E = 16, 128, 512, 64
P = 512            # output projection size (== Q here)
N_CORES = 8
E_LOC = E // N_CORES   # 8 channels per core
M = B * C              # 2048 rows per channel GEMM

QT = Q // 128          # 4 k-tiles
PT = P // 128          # 4 output-partition tiles
MC = M // 512          # 4 moving-dim chunks

BF16 = mybir.dt.bfloat16
F32 = mybir.dt.float32

_CACHE = {}


def _kernel_body(tc, out, xt, wt, bias_d):
    nc = tc.nc
    with ExitStack() as ctx:
        xpool = ctx.enter_context(tc.tile_pool(name="x", bufs=3))
        wpool = ctx.enter_context(tc.tile_pool(name="w", bufs=3))
        opool = ctx.enter_context(tc.tile_pool(name="o", bufs=6))
        bpool = ctx.enter_context(tc.tile_pool(name="bias", bufs=1))
        psum = ctx.enter_context(tc.tile_pool(name="psum", bufs=2, space="PSUM"))

        bias_sb = bpool.tile([128, E_LOC * PT], F32)

        def mm_group(ps, w_sb, x_sb, pt, mc):
            for qt in range(QT):
                nc.tensor.matmul(
                    ps[:],
                    w_sb[:, qt, ts(pt, 128)],
                    x_sb[:, qt, ts(mc, 512)],
                    start=(qt == 0),
                    stop=(qt == QT - 1),
                )

        def evict(ps, o_ap, bias_ap, parity):
            # alternate eviction between ACT and DVE so neither is the
            # bottleneck
            if parity % 2 == 0:
                nc.scalar.activation(
                    o_ap,
                    ps[:],
                    mybir.ActivationFunctionType.Identity,
                    bias=bias_ap,
                )
            else:
                nc.vector.tensor_scalar_add(o_ap, ps[:], bias_ap)

        for e in range(E_LOC):
            # whole channel of W^T first (small, needed by every matmul)
            w_sb = wpool.tile([128, QT, P], BF16, tag="w")
            nc.sync.dma_start(
                w_sb[:], wt[e].rearrange("(qt qp) p -> qp qt p", qp=128)
            )
            # channel of X^T, one DMA per m-chunk so compute can start after
            # the first 512 KiB instead of the full 2 MiB
            x_sb = xpool.tile([128, QT, M], BF16, tag="x")
            xt_r = xt[e].rearrange("(qt qp) m -> qp qt m", qp=128)
            for mc in range(MC):
                nc.sync.dma_start(
                    x_sb[:, :, ts(mc, 512)], xt_r[:, :, ts(mc, 512)]
                )
                if e == 0 and mc == 0:
                    # bias is first needed by the first eviction; issuing it
                    # after w + x-chunk-0 keeps it off the startup critical
                    # path
                    nc.sync.dma_start(bias_sb[:], bias_d[:])

            if e == 0:
                # Channel 0 runs mc-outer so each x chunk feeds 16 matmuls
                # (~3.4 us) while the next 512 KiB chunk streams in (~1.5 us)
                # -- the PE never outruns the startup DMA.
                o_sbs = [
                    opool.tile([128, M], BF16, tag="o", name=f"o_{pt}")
                    for pt in range(PT)
                ]
                for mc in range(MC):
                    for pt in range(PT):
                        ps = psum.tile([128, 512], F32, name=f"ps_{pt % 2}")
                        mm_group(ps, w_sb, x_sb, pt, mc)
                        evict(
                            ps,
                            o_sbs[pt][:, ts(mc, 512)],
                            bias_sb[:, e * PT + pt : e * PT + pt + 1],
                            mc + pt,
                        )
                for pt in range(PT):
                    # out DMA rides the ACT HWDGE ring so its eviction-gated
                    # wait never blocks the Sync ring's input prefetch
                    # (HWDGE is FIFO per ring)
                    nc.scalar.dma_start(out[e, ts(pt, 128)], o_sbs[pt][:])
                continue

            last_ch = e == E_LOC - 1
            for pt in range(PT):
                bj = e * PT + pt
                bias_ap = bias_sb[:, bj : bj + 1]
                last_pt = last_ch and pt == PT - 1
                o_sb = opool.tile([128, M], BF16, tag="o")
                for mc in range(MC):
                    # mc-outer / qt-inner: each PSUM bank's accumulation
                    # group is 4 back-to-back matmuls, so bank mc evicts
                    # while bank mc+1 computes
                    ps = psum.tile([128, 512], F32, name=f"ps_{mc % 2}")
                    mm_group(ps, w_sb, x_sb, pt, mc)
                    evict(ps, o_sb[:, ts(mc, 512)], bias_ap, mc)
                    if last_pt:
                        # pipeline the final tile's writeback per chunk so
                        # the kernel tail only waits on the last 128 KiB
                        nc.scalar.dma_start(
                            out[e, ts(pt, 128), ts(mc, 512)],
                            o_sb[:, ts(mc, 512)],
                        )
                if not last_pt:
                    nc.scalar.dma_start(out[e, ts(pt, 128)], o_sb[:])


def _build():
    if "nc" in _CACHE:
        return _CACHE["nc"]
    nc = bacc.Bacc(
        "TRN2",
        target_bir_lowering=False,
        debug=False,
        enable_asserts=True,
        num_devices=N_CORES,
        enable_partition_id=False,
    )
    xt = nc.dram_tensor("xt", [E_LOC, Q, M], BF16, kind="ExternalInput").ap()
    wt = nc.dram_tensor("wt", [E_LOC, Q, P], BF16, kind="ExternalInput").ap()
    bias_d = nc.dram_tensor("bias", [128, E_LOC * PT], F32, kind="ExternalInput").ap()
    out = nc.dram_tensor("out", [E_LOC, P, M], BF16, kind="ExternalOutput").ap()
    with tile.TileContext(nc) as tc:
        _kernel_body(tc, out, xt, wt, bias_d)
    nc.compile()
    _CACHE["nc"] = nc
    return nc


def make_in_maps(x, W, b):
    """Host-side shard + cast + layout. Returns list of 8 per-core dicts."""
    in_maps = []
    for r in range(N_CORES):
        e0 = r * E_LOC
        e1 = e0 + E_LOC
        # X^T per channel: [e, q, m] where m = b*C + c
        xs = (
            x[:, :, :, e0:e1]
            .transpose(3, 2, 0, 1)
            .astype(ml_dtypes.bfloat16)
            .reshape(E_LOC, Q, M)
        )
        # W^T per channel: [e, q, p]
        ws = np.ascontiguousarray(W[e0:e1].transpose(0, 2, 1)).astype(
            ml_dtypes.bfloat16
        )
        # bias laid out [128, e*PT + pt] with partition = p % 128
        bs = np.ascontiguousarray(
            b[e0:e1].reshape(E_LOC, PT, 128).transpose(2, 0, 1)
        ).reshape(128, E_LOC * PT)
        in_maps.append({"xt": xs, "wt": ws, "bias": np.ascontiguousarray(bs, np.float32)})
    return in_maps


def assemble_output(results):
    out = np.empty((B, C, P, E), np.float32)
    for r in range(N_CORES):
        o = results[r]["out"]  # [E_LOC, P, M] bf16
        out[:, :, :, r * E_LOC : (r + 1) * E_LOC] = (
            o.astype(np.float32).transpose(2, 1, 0).reshape(B, C, P, E_LOC)
        )
    return out


def run_on_hw(x, W, b, **spmd_kwargs):
    nc = _build()
    in_maps = make_in_maps(x, W, b)
    res = bass_utils.run_bass_kernel_spmd(
        nc, in_maps, core_ids=list(range(N_CORES)), **spmd_kwargs
    )
    return assemble_output(res.results), res


def kernel(x, W, b):
    out, _ = run_on_hw(x, W, b)
    return out


# revision 8
# speedup vs baseline: 1.3267x; 1.0097x over previous
"""Trainium2 Bass kernel for per-channel batched Linear:
    out[b,c,p,e] = sum_q W[e,p,q] * x[b,c,q,e] + bias[e,p]

Shapes: x [16,128,512,64] f32, W [64,512,512] f32, b [64,512] f32.

Strategy: shard embed_dim E=64 across 8 cores (8 channels/core). Each
channel is an independent GEMM out_e^T = W_e @ X_e^T with X_e = x[..,..,:,e]
flattened to [2048, 512]. We compute the transposed output [P, M] so the
weight is the matmul's stationary operand (reused across the M dim) and the
bias becomes a per-partition scalar for the PSUM->SBUF eviction op.

Host side: slice + cast fp32 -> bf16 + lay out operands so every DMA is a
big contiguous 128-partition transfer. Device: bf16 matmuls (full PE rate),
fp32 PSUM accumulate, bias added during PSUM eviction on ACT/DVE.
"""

import numpy as np
import ml_dtypes
from contextlib import ExitStack

import concourse.bass as bass
import concourse.tile as tile
from concourse import bacc, mybir
from concourse import bass_utils
from concourse.bass import ts

B, C, Q, E = 16, 128, 512, 64
P = 512            # output projection size (== Q here)
N_CORES = 8
E_LOC = E // N_CORES   # 8 channels per core
M = B * C              # 2048 rows per channel GEMM

QT = Q // 128          # 4 k-tiles
PT = P // 128          # 4 output-partition tiles
MC = M // 512          # 4 moving-dim chunks

BF16 = mybir.dt.bfloat16
F32 = mybir.dt.float32

_CACHE = {}


def _kernel_body(tc, out, xt, wt, bias_d):
    nc = tc.nc
    with ExitStack() as ctx:
        xpool = ctx.enter_context(tc.tile_pool(name="x", bufs=3))
        wpool = ctx.enter_context(tc.tile_pool(name="w", bufs=3))
        opool = ctx.enter_context(tc.tile_pool(name="o", bufs=6))
        bpool = ctx.enter_context(tc.tile_pool(name="bias", bufs=1))
        psum = ctx.enter_context(tc.tile_pool(name="psum", bufs=2, space="PSUM"))

        bias_sb = bpool.tile([128, E_LOC * PT], F32)

        # ~48 throwaway matmuls with no data dependencies: they run during
        # the input-DMA wait and push the PE's HAM activity monitor to
        # K=8/8 (2.4 GHz) before the first real matmul arrives.
        warm_src = bpool.tile([128, 128], BF16, name="warm_src")
        nc.gpsimd.memset(warm_src[:], 0.0)
        warm_ps = psum.tile([128, 128], F32, tag="ps_0", name="warm_ps")
        for _ in range(48):
            nc.tensor.matmul(
                warm_ps[:], warm_src[:], warm_src[:], start=True, stop=True
            )

        def mm_group(ps, lhsT_of_qt, x_sb, mc):
            for qt in range(QT):
                nc.tensor.matmul(
                    ps[:],
                    lhsT_of_qt(qt),
                    x_sb[:, qt, ts(mc, 512)],
                    start=(qt == 0),
                    stop=(qt == QT - 1),
                )

        def evict(ps, o_ap, bias_ap):
            # evictions live on DVE only: the ACT engine issues the out
            # DMAs, and a DMA-issue busy ACT must never delay the PSUM
            # WAR-release
            nc.vector.tensor_scalar_add(o_ap, ps[:], bias_ap)

        for e in range(E_LOC):
            if e == 0:
                # channel 0: per-p-tile w DMAs so the first matmul group
                # only waits on 128 KiB of weights instead of 512 KiB
                w_sb = wpool.tile([128, QT, P], BF16, tag="w", name="w_sb")
                for pt in range(PT):
                    nc.sync.dma_start(
                        w_sb[:, :, ts(pt, 128)],
                        wt[e][:, ts(pt, 128)].rearrange(
                            "(qt qp) p -> qp qt p", qp=128
                        ),
                    )
            else:
                # whole channel of W^T in one DMA (small, prefetched early)
                w_sb = wpool.tile([128, QT, P], BF16, tag="w", name="w_sb")
                nc.sync.dma_start(
                    w_sb[:], wt[e].rearrange("(qt qp) p -> qp qt p", qp=128)
                )
            # channel of X^T, one DMA per m-chunk so compute can start after
            # the first 512 KiB instead of the full 2 MiB
            x_sb = xpool.tile([128, QT, M], BF16, tag="x")
            xt_r = xt[e].rearrange("(qt qp) m -> qp qt m", qp=128)
            for mc in range(MC):
                nc.sync.dma_start(
                    x_sb[:, :, ts(mc, 512)], xt_r[:, :, ts(mc, 512)]
                )
                if e == 0 and mc == 0:
                    # bias is first needed by the first eviction; issuing it
                    # after w + x-chunk-0 keeps it off the startup critical
                    # path
                    nc.sync.dma_start(bias_sb[:], bias_d[:])

            def lhsT_of_qt(qt, w_sb=w_sb, pt=0):
                return w_sb[:, qt, ts(pt, 128)]

            if e == 0:
                # Channel 0 runs mc-outer so each x chunk feeds 16 matmuls
                # (~3.4 us) while the next 512 KiB chunk streams in (~1.5 us)
                # -- the PE never outruns the startup DMA.
                o_sbs = [
                    opool.tile([128, M], BF16, tag="o", name=f"o_{pt}")
                    for pt in range(PT)
                ]
                for mc in range(MC):
                    for pt in range(PT):
                        ps = psum.tile(
                            [128, 512], F32, tag=f"ps_{pt}", name=f"ps_{pt}"
                        )
                        mm_group(
                            ps,
                            lambda qt, pt=pt: w_sb[:, qt, ts(pt, 128)],
                            x_sb,
                            mc,
                        )
                        evict(
                            ps,
                            o_sbs[pt][:, ts(mc, 512)],
                            bias_sb[:, e * PT + pt : e * PT + pt + 1],
                        )
                for pt in range(PT):
                    # out DMA rides the ACT HWDGE ring so its eviction-gated
                    # wait never blocks the Sync ring's input prefetch
                    # (HWDGE is FIFO per ring)
                    nc.scalar.dma_start(out[e, ts(pt, 128)], o_sbs[pt][:])
                continue

            last_ch = e == E_LOC - 1
            for pt in range(PT):
                bj = e * PT + pt
                bias_ap = bias_sb[:, bj : bj + 1]
                last_pt = last_ch and pt == PT - 1
                o_sb = opool.tile([128, M], BF16, tag="o")
                for mc in range(MC):
                    # mc-outer / qt-inner: each PSUM bank's accumulation
                    # group is 4 back-to-back matmuls, so bank mc evicts
                    # while bank mc+1 computes; 4 tags x 2 bufs = all 8
                    # PSUM banks rotate with reuse distance 8 groups
                    ps = psum.tile(
                        [128, 512], F32, tag=f"ps_{mc}", name=f"ps_{mc}"
                    )
                    mm_group(
                        ps,
                        lambda qt, pt=pt: w_sb[:, qt, ts(pt, 128)],
                        x_sb,
                        mc,
                    )
                    evict(ps, o_sb[:, ts(mc, 512)], bias_ap)
                    if last_pt:
                        # pipeline the final tile's writeback per chunk so
                        # the kernel tail only waits on the last 128 KiB
                        nc.scalar.dma_start(
                            out[e, ts(pt, 128), ts(mc, 512)],
                            o_sb[:, ts(mc, 512)],
                        )
                if not last_pt:
                    nc.scalar.dma_start(out[e, ts(pt, 128)], o_sb[:])


def _build():
    if "nc" in _CACHE:
        return _CACHE["nc"]
    nc = bacc.Bacc(
        "TRN2",
        target_bir_lowering=False,
        debug=False,
        enable_asserts=True,
        num_devices=N_CORES,
        enable_partition_id=False,
    )
    xt = nc.dram_tensor("xt", [E_LOC, Q, M], BF16, kind="ExternalInput").ap()
    wt = nc.dram_tensor("wt", [E_LOC, Q, P], BF16, kind="ExternalInput").ap()
    bias_d = nc.dram_tensor("bias", [128, E_LOC * PT], F32, kind="ExternalInput").ap()
    out = nc.dram_tensor("out", [E_LOC, P, M], BF16, kind="ExternalOutput").ap()
    with tile.TileContext(nc) as tc:
        _kernel_body(tc, out, xt, wt, bias_d)
    nc.compile()
    _CACHE["nc"] = nc
    return nc


def make_in_maps(x, W, b):
    """Host-side shard + cast + layout. Returns list of 8 per-core dicts."""
    in_maps = []
    for r in range(N_CORES):
        e0 = r * E_LOC
        e1 = e0 + E_LOC
        # X^T per channel: [e, q, m] where m = b*C + c
        xs = (
            x[:, :, :, e0:e1]
            .transpose(3, 2, 0, 1)
            .astype(ml_dtypes.bfloat16)
            .reshape(E_LOC, Q, M)
        )
        # W^T per channel: [e, q, p]
        ws = np.ascontiguousarray(W[e0:e1].transpose(0, 2, 1)).astype(
            ml_dtypes.bfloat16
        )
        # bias laid out [128, e*PT + pt] with partition = p % 128
        bs = np.ascontiguousarray(
            b[e0:e1].reshape(E_LOC, PT, 128).transpose(2, 0, 1)
        ).reshape(128, E_LOC * PT)
        in_maps.append({"xt": xs, "wt": ws, "bias": np.ascontiguousarray(bs, np.float32)})
    return in_maps


def assemble_output(results):
    out = np.empty((B, C, P, E), np.float32)
    for r in range(N_CORES):
        o = results[r]["out"]  # [E_LOC, P, M] bf16
        out[:, :, :, r * E_LOC : (r + 1) * E_LOC] = (
            o.astype(np.float32).transpose(2, 1, 0).reshape(B, C, P, E_LOC)
        )
    return out


def run_on_hw(x, W, b, **spmd_kwargs):
    nc = _build()
    in_maps = make_in_maps(x, W, b)
    res = bass_utils.run_bass_kernel_spmd(
        nc, in_maps, core_ids=list(range(N_CORES)), **spmd_kwargs
    )
    return assemble_output(res.results), res


def kernel(x, W, b):
    out, _ = run_on_hw(x, W, b)
    return out


# revision 10
# speedup vs baseline: 1.3545x; 1.0210x over previous
"""Trainium2 Bass kernel for per-channel batched Linear:
    out[b,c,p,e] = sum_q W[e,p,q] * x[b,c,q,e] + bias[e,p]

Shapes: x [16,128,512,64] f32, W [64,512,512] f32, b [64,512] f32.

Strategy: shard embed_dim E=64 across 8 cores (8 channels/core). Each
channel is an independent GEMM out_e^T = W_e @ X_e^T with X_e = x[..,..,:,e]
flattened to [2048, 512]. We compute the transposed output [P, M] so the
weight is the matmul's stationary operand (reused across the M dim) and the
bias becomes a per-partition scalar for the PSUM->SBUF eviction op.

Host side: slice + cast fp32 -> bf16 + lay out operands so every DMA is a
big contiguous 128-partition transfer. Device: bf16 matmuls (full PE rate),
fp32 PSUM accumulate, bias added during PSUM eviction on ACT/DVE.
"""

import numpy as np
import ml_dtypes
from contextlib import ExitStack

import concourse.bass as bass
import concourse.tile as tile
from concourse import bacc, mybir
from concourse import bass_utils
from concourse.bass import ts

B, C, Q, E = 16, 128, 512, 64
P = 512            # output projection size (== Q here)
N_CORES = 8
E_LOC = E // N_CORES   # 8 channels per core
M = B * C              # 2048 rows per channel GEMM

QT = Q // 128          # 4 k-tiles
PT = P // 128          # 4 output-partition tiles
MC = M // 512          # 4 moving-dim chunks

BF16 = mybir.dt.bfloat16
F32 = mybir.dt.float32

_CACHE = {}


def _kernel_body(tc, out, xt, wt, bias_d):
    nc = tc.nc
    with ExitStack() as ctx:
        xpool = ctx.enter_context(tc.tile_pool(name="x", bufs=3))
        wpool = ctx.enter_context(tc.tile_pool(name="w", bufs=3))
        opool = ctx.enter_context(tc.tile_pool(name="o", bufs=6))
        bpool = ctx.enter_context(tc.tile_pool(name="bias", bufs=1))
        psum = ctx.enter_context(tc.tile_pool(name="psum", bufs=2, space="PSUM"))

        bias_sb = bpool.tile([128, E_LOC * PT], F32)

        # ~48 throwaway matmuls with no data dependencies: they run during
        # the input-DMA wait and push the PE's HAM activity monitor to
        # K=8/8 (2.4 GHz) before the first real matmul arrives.
        warm_src = bpool.tile([128, 128], BF16, name="warm_src")
        nc.gpsimd.memset(warm_src[:], 0.0)
        warm_ps = psum.tile([128, 128], F32, tag="ps_0", name="warm_ps")
        for _ in range(52):
            nc.tensor.matmul(
                warm_ps[:], warm_src[:], warm_src[:], start=True, stop=True
            )

        def mm_group(ps, lhsT_of_qt, x_sb, mc):
            for qt in range(QT):
                nc.tensor.matmul(
                    ps[:],
                    lhsT_of_qt(qt),
                    x_sb[:, qt, ts(mc, 512)],
                    start=(qt == 0),
                    stop=(qt == QT - 1),
                )

        def evict(ps, o_ap, bias_ap):
            # evictions live on DVE only: the ACT engine issues the out
            # DMAs, and a DMA-issue busy ACT must never delay the PSUM
            # WAR-release
            nc.vector.tensor_scalar_add(o_ap, ps[:], bias_ap)

        for e in range(E_LOC):
            w_sb = wpool.tile([128, QT, P], BF16, tag="w", name="w_sb")
            x_sb = xpool.tile([128, QT, M], BF16, tag="x")
            xt_r = xt[e].rearrange("(qt qp) m -> qp qt m", qp=128)
            wt_r = wt[e].rearrange("(qt qp) p -> qp qt p", qp=128)
            if e == 0:
                # Startup-critical issue order: the first matmul group needs
                # only w's pt0 columns (128 KiB) + x chunk 0 (512 KiB), so
                # those two go first; everything else streams behind them.
                nc.sync.dma_start(
                    w_sb[:, :, ts(0, 128)],
                    wt[e][:, ts(0, 128)].rearrange("(qt qp) p -> qp qt p", qp=128),
                )
                nc.sync.dma_start(x_sb[:, :, ts(0, 512)], xt_r[:, :, ts(0, 512)])
                nc.sync.dma_start(bias_sb[:], bias_d[:])
                for pt in range(1, PT):
                    nc.sync.dma_start(
                        w_sb[:, :, ts(pt, 128)],
                        wt[e][:, ts(pt, 128)].rearrange(
                            "(qt qp) p -> qp qt p", qp=128
                        ),
                    )
                for mc in range(1, MC):
                    nc.sync.dma_start(
                        x_sb[:, :, ts(mc, 512)], xt_r[:, :, ts(mc, 512)]
                    )
            else:
                # steady state: whole w in one DMA, x per m-chunk
                nc.sync.dma_start(w_sb[:], wt_r)
                for mc in range(MC):
                    nc.sync.dma_start(
                        x_sb[:, :, ts(mc, 512)], xt_r[:, :, ts(mc, 512)]
                    )

            def lhsT_of_qt(qt, w_sb=w_sb, pt=0):
                return w_sb[:, qt, ts(pt, 128)]

            if e == 0:
                # Channel 0 runs mc-outer so each x chunk feeds 16 matmuls
                # (~3.4 us) while the next 512 KiB chunk streams in (~1.5 us)
                # -- the PE never outruns the startup DMA.
                o_sbs = [
                    opool.tile([128, M], BF16, tag="o", name=f"o_{pt}")
                    for pt in range(PT)
                ]
                for mc in range(MC):
                    for pt in range(PT):
                        ps = psum.tile(
                            [128, 512], F32, tag=f"ps_{pt}", name=f"ps_{pt}"
                        )
                        mm_group(
                            ps,
                            lambda qt, pt=pt: w_sb[:, qt, ts(pt, 128)],
                            x_sb,
                            mc,
                        )
                        evict(
                            ps,
                            o_sbs[pt][:, ts(mc, 512)],
                            bias_sb[:, e * PT + pt : e * PT + pt + 1],
                        )
                for pt in range(PT):
                    # out DMA rides the ACT HWDGE ring so its eviction-gated
                    # wait never blocks the Sync ring's input prefetch
                    # (HWDGE is FIFO per ring)
                    nc.scalar.dma_start(out[e, ts(pt, 128)], o_sbs[pt][:])
                continue

            last_ch = e == E_LOC - 1
            for pt in range(PT):
                bj = e * PT + pt
                bias_ap = bias_sb[:, bj : bj + 1]
                last_pt = last_ch and pt == PT - 1
                o_sb = opool.tile([128, M], BF16, tag="o")
                for mc in range(MC):
                    # mc-outer / qt-inner: each PSUM bank's accumulation
                    # group is 4 back-to-back matmuls, so bank mc evicts
                    # while bank mc+1 computes; 4 tags x 2 bufs = all 8
                    # PSUM banks rotate with reuse distance 8 groups
                    ps = psum.tile(
                        [128, 512], F32, tag=f"ps_{mc}", name=f"ps_{mc}"
                    )
                    mm_group(
                        ps,
                        lambda qt, pt=pt: w_sb[:, qt, ts(pt, 128)],
                        x_sb,
                        mc,
                    )
                    evict(ps, o_sb[:, ts(mc, 512)], bias_ap)
                    if last_pt:
                        # pipeline the final tile's writeback per chunk so
                        # the kernel tail only waits on the last 128 KiB
                        nc.scalar.dma_start(
                            out[e, ts(pt, 128), ts(mc, 512)],
                            o_sb[:, ts(mc, 512)],
                        )
                if not last_pt:
                    nc.scalar.dma_start(out[e, ts(pt, 128)], o_sb[:])


def _build():
    if "nc" in _CACHE:
        return _CACHE["nc"]
    nc = bacc.Bacc(
        "TRN2",
        target_bir_lowering=False,
        debug=False,
        enable_asserts=True,
        num_devices=N_CORES,
        enable_partition_id=False,
    )
    xt = nc.dram_tensor("xt", [E_LOC, Q, M], BF16, kind="ExternalInput").ap()
    wt = nc.dram_tensor("wt", [E_LOC, Q, P], BF16, kind="ExternalInput").ap()
    bias_d = nc.dram_tensor("bias", [128, E_LOC * PT], F32, kind="ExternalInput").ap()
    out = nc.dram_tensor("out", [E_LOC, P, M], BF16, kind="ExternalOutput").ap()
    with tile.TileContext(nc) as tc:
        _kernel_body(tc, out, xt, wt, bias_d)
    nc.compile()
    _CACHE["nc"] = nc
    return nc


def make_in_maps(x, W, b):
    """Host-side shard + cast + layout. Returns list of 8 per-core dicts."""
    in_maps = []
    for r in range(N_CORES):
        e0 = r * E_LOC
        e1 = e0 + E_LOC
        # X^T per channel: [e, q, m] where m = b*C + c
        xs = (
            x[:, :, :, e0:e1]
            .transpose(3, 2, 0, 1)
            .astype(ml_dtypes.bfloat16)
            .reshape(E_LOC, Q, M)
        )
        # W^T per channel: [e, q, p]
        ws = np.ascontiguousarray(W[e0:e1].transpose(0, 2, 1)).astype(
            ml_dtypes.bfloat16
        )
        # bias laid out [128, e*PT + pt] with partition = p % 128
        bs = np.ascontiguousarray(
            b[e0:e1].reshape(E_LOC, PT, 128).transpose(2, 0, 1)
        ).reshape(128, E_LOC * PT)
        in_maps.append({"xt": xs, "wt": ws, "bias": np.ascontiguousarray(bs, np.float32)})
    return in_maps


def assemble_output(results):
    out = np.empty((B, C, P, E), np.float32)
    for r in range(N_CORES):
        o = results[r]["out"]  # [E_LOC, P, M] bf16
        out[:, :, :, r * E_LOC : (r + 1) * E_LOC] = (
            o.astype(np.float32).transpose(2, 1, 0).reshape(B, C, P, E_LOC)
        )
    return out


def run_on_hw(x, W, b, **spmd_kwargs):
    nc = _build()
    in_maps = make_in_maps(x, W, b)
    res = bass_utils.run_bass_kernel_spmd(
        nc, in_maps, core_ids=list(range(N_CORES)), **spmd_kwargs
    )
    return assemble_output(res.results), res


def kernel(x, W, b):
    out, _ = run_on_hw(x, W, b)
    return out
